# revision 29
# baseline (speedup 1.0000x reference)
"""BitLinear (fake-quant straight-through) Trainium2 kernel.

Math (per the reference nn module):
  dqx = round(x * s_x) / s_x       s_x = 127 / clip(rowabsmax(x), 1e-5)  (per token)
  dqw = clip(round(w * s_w), -1, 1) / s_w   s_w = 1 / clip(mean|w|, 1e-5) (per tensor)
  out = dqx @ dqw.T + bias

Design:
  * round(x*s_x) is an integer in [-127,127] and the ternary weight is in
    {-1,0,1}; both are EXACT in bf16 and the matmul accumulates exactly in
    fp32 PSUM, so the heavy matmul runs at full bf16 PE rate.  round() is
    the fp32-RNE magic-constant trick (v + 1.5*2^23) - 1.5*2^23.
  * Host-side input prep (a fraction of a percent of the matmul FLOPs,
    bit-exact fp32 numpy matching the reference rounding): the weight is
    ternary-quantized + transposed; the activations are quantized to int8
    (xq = round(x*ss) from the exact f32 x) with the per-token output
    scale fs shipped as a tiny side tensor.  x ships as int8 (4 MiB) and
    out returns as bf16 (8 MiB), so HBM traffic is ~18 MiB/core vs 36 for
    the naive f32 path; measured rel-err 2.1e-3 vs the 2e-2 gate (bf16
    output rounding is the only approximation).
  * Per 512-token quad: int8 x load (scalar/ACT HWDGE ring) -> ACT
    int8->bf16 widen (never gpsimd: its ->bf16 write-cast ucode is ~50x
    slow) -> one xbar transpose (sync/SP ring) -> 64 back-to-back
    512-wide bf16 matmuls -> DVE scalar_tensor_tensor fused evac
    bf16(psum*fs + bias) -> SWDGE store (gpsimd ring).  Every pipeline
    stage owns one engine and one DMA ring, so stages only queue behind
    themselves.
  * Tile's xbar-hang workaround makes each DMA transpose wait for ALL
    earlier-scheduled DMA copies; add_dep_helper pins load(q) after
    transpose(q-2) in the schedule so transposes never stall on far-future
    loads (worth ~25 us end-to-end).

Sharding: data parallel over batch; core i computes batch element i with
the full weight.  No collectives; the host scatters x / gathers out.
"""

import numpy as np

from concourse import bacc, bass, mybir, tile
from concourse.bass_utils import run_bass_kernel_spmd
from concourse.tile_rust import add_dep_helper

F32 = mybir.dt.float32
FP16 = mybir.dt.float16
BF16 = mybir.dt.bfloat16
INT8 = mybir.dt.int8
ALU = mybir.AluOpType
ACTF = mybir.ActivationFunctionType

MAGIC = 12582912.0  # 1.5 * 2**23: fp32 RNE round-to-integer constant
EPS = 1e-05

B, S, K, N = 8, 4096, 1024, 1024
N_CORES = 8
QS = 4  # token tiles per quad


def build(s_tokens=S, k=K, n=N):
    nc = bacc.Bacc("TRN2", target_bir_lowering=False, debug=False)

    KT = k // 128
    NT = n // 128
    NH = n // 512
    NQ = s_tokens // (128 * QS)
    NC = NQ * QS  # scale columns

    x_d = nc.dram_tensor("x", [s_tokens, k], INT8, kind="ExternalInput").ap()
    qwt_d = nc.dram_tensor("qwt", [128, NT, KT, 128], BF16, kind="ExternalInput").ap()
    bias_d = nc.dram_tensor("biasb", [128, n], F32, kind="ExternalInput").ap()
    # scales[p, 0:NC] = fs (output scale) per token
    scales_d = nc.dram_tensor("scales", [128, NC], F32, kind="ExternalInput").ap()
    out_d = nc.dram_tensor("out", [s_tokens, n], BF16, kind="ExternalOutput").ap()

    x_q = x_d.rearrange("(q s p) k -> q p s k", s=QS, p=128)
    out_q = out_d.rearrange("(q s p) n -> q p s n", s=QS, p=128)

    with tile.TileContext(nc) as tc:
        with (
            tc.tile_pool(name="static", bufs=1) as static,
            tc.tile_pool(name="xpool", bufs=5) as xpool,
            tc.tile_pool(name="qpool", bufs=3) as qpool,
            tc.tile_pool(name="qtpool", bufs=3) as qtpool,
            tc.tile_pool(name="opool", bufs=3) as opool,
            tc.tile_pool(name="psum", bufs=3, space="PSUM") as psum_pool,
        ):
            scales = static.tile([128, NC], F32)
            nc.gpsimd.dma_start(scales[:], scales_d[:])
            # weight in two half-tiles: h=0 matmuls need only nt 0-3, so
            # the first matmul waits on 1 MiB, not 2; the second half and
            # the bias are pinned after transpose(q0) so the xbar-hang
            # serialization can't gate that transpose on them.
            qwT_h = [static.tile([128, NT // 2, KT, 128], BF16, name=f"qwT{h}")
                     for h in range(2)]
            nc.gpsimd.dma_start(qwT_h[0][:], qwt_d[:, 0:NT // 2])
            qwtb_inst = nc.gpsimd.dma_start(qwT_h[1][:], qwt_d[:, NT // 2:NT])
            bias_sb = static.tile([128, n], F32)
            bias_inst = nc.gpsimd.dma_start(bias_sb[:], bias_d[:])

            transp_insts = []
            for q in range(NQ):
                x_s = xpool.tile([128, QS, k], INT8, name="x_s")
                load_inst = nc.scalar.dma_start(x_s[:], x_q[q])
                if q >= 2:
                    # schedule-order pin: Tile's xbar-hang workaround makes
                    # every DMA transpose wait for ALL earlier-scheduled DMA
                    # copies; without this pin the scheduler hoists far-
                    # future x loads ahead of transpose(q-2), which then
                    # stalls on them.
                    add_dep_helper(
                        load_inst.ins, transp_insts[q - 2].ins, sync=False,
                        reason="keep load(q) after transpose(q-2) in schedule",
                    )

                # int8 -> bf16 widen on ACT (x ships pre-quantized; the
                # int values [-127,127] are exact in bf16)
                qx = qpool.tile([128, QS, k], BF16, name="qx")
                nc.scalar.activation(qx[:], x_s[:], ACTF.Copy)

                # one xbar transpose for the whole quad
                qxT = qtpool.tile([128, QS, KT, 128], BF16, name="qxT")
                t_inst = nc.sync.dma_start_transpose(qxT[:], qx[:])
                transp_insts.append(t_inst)
                if q == 0:
                    for late in (qwtb_inst, bias_inst):
                        add_dep_helper(
                            late.ins, t_inst.ins, sync=False,
                            reason="late statics after transpose(q0)",
                        )

                outs = opool.tile([128, QS, n], BF16, name="outs")
                for s in range(QS):
                    col = q * QS + s
                    ps_list = [
                        psum_pool.tile([128, 512], F32, name=f"ps{h}", tag=f"ps{h}")
                        for h in range(NH)
                    ]
                    # h-blocked so the h=0 group depends only on the
                    # first weight half-tile, and its evac fires before the
                    # h=1 group finishes
                    for h in range(NH):
                        for kt in range(KT):
                            nc.tensor.matmul(
                                ps_list[h][:],
                                qxT[:, s, kt, :],
                                qwT_h[h][:, :, kt, :],
                                start=(kt == 0),
                                stop=(kt == KT - 1),
                            )
                        # fused evac: outs = bf16(psum * fs[s] + bias)
                        nc.vector.scalar_tensor_tensor(
                            outs[:, s, h * 512:(h + 1) * 512],
                            ps_list[h][:],
                            scales[:, col:col + 1],
                            bias_sb[:, h * 512:(h + 1) * 512],
                            ALU.mult,
                            ALU.add,
                        )
                nc.gpsimd.dma_start(out_q[q], outs[:])

    nc.compile()
    return nc


def host_weight(weight):
    import ml_dtypes

    w = np.ascontiguousarray(weight, dtype=np.float32)
    try:
        import jax
        import jax.numpy as jnp

        with jax.default_device(jax.devices("cpu")[0]):
            mean_abs = np.float32(
                jax.device_get(jnp.mean(jnp.abs(jnp.asarray(w, dtype=jnp.float32))))
            )
    except Exception:
        mean_abs = np.float32(np.mean(np.abs(w), dtype=np.float32))
    mean_c = np.maximum(mean_abs, np.float32(EPS))
    sw = np.float32(1.0) / mean_c
    tern = np.clip(np.rint(w * sw), -1.0, 1.0).astype(ml_dtypes.bfloat16)
    NT, KT = N // 128, K // 128
    qwt = np.ascontiguousarray(
        tern.reshape(NT, 128, KT, 128).transpose(3, 0, 2, 1)
    )
    wdiv = np.float32(1.0) / sw
    k1 = wdiv / np.float32(127.0)
    return qwt, k1


def host_quant(x_core, k1):
    """Pre-quantize activations bit-exactly like the reference: int8
    xq = round(x*ss) from the exact f32 x, plus the per-token output
    scale fs laid out as scales[p, q*QS + s] for token t = q*512+s*128+p."""
    cc = np.maximum(
        np.abs(x_core).max(axis=1), np.float32(EPS)
    ).astype(np.float32)                       # [s_tokens]
    ssv = np.float32(127.0) / cc               # one division, like the reference
    xq = np.clip(np.rint(x_core * ssv[:, None]), -127, 127).astype(np.int8)
    fsv = cc * np.float32(k1)
    NQ = x_core.shape[0] // 512
    fs_t = fsv.reshape(NQ * QS, 128).T         # [128, NQ*QS]
    return xq, np.ascontiguousarray(fs_t, dtype=np.float32)


def make_in_maps(x, weight, bias):
    x = np.ascontiguousarray(x, dtype=np.float32)
    bias = np.ascontiguousarray(bias, dtype=np.float32)
    qwt, k1 = host_weight(weight)
    biasb = np.tile(bias[None, :], (128, 1)).copy()
    maps = []
    for i in range(N_CORES):
        xq, fs = host_quant(x[i], k1)
        maps.append({"x": xq, "qwt": qwt, "biasb": biasb, "scales": fs})
    return maps


_NC_CACHE = {}


def _get_nc():
    if "nc" not in _NC_CACHE:
        _NC_CACHE["nc"] = build()
    return _NC_CACHE["nc"]


def kernel(x, weight, bias, **kwargs):
    nc = _get_nc()
    in_maps = make_in_maps(x, weight, bias)
    last_err = None
    for _attempt in range(3):
        try:
            res = run_bass_kernel_spmd(nc, in_maps, list(range(N_CORES)))
            return np.stack(
                [
                    np.asarray(res.results[i]["out"]).astype(np.float32)
                    for i in range(N_CORES)
                ],
                axis=0,
            )
        except Exception as e:  # transient NRT device errors: retry
            last_err = e
    raise last_err
